# revision 46
# baseline (speedup 1.0000x reference)
import sys
sys.path.insert(0, "/opt/trn_rl_repo")
import numpy as np

B, T, C, H = 2, 2048, 1024, 16
D = C // H          # 64
HPC = 4             # heads per core
OC = HPC * D        # 256 v-channels per core
NT = T // 512       # 4 query windows / t-blocks
NCH = C // 128      # 8 contraction chunks
NEG = -1e30
LN4 = 1.3862943611198906  # exp bias: est = exp(s/8)/4 keeps fp8e4 range safe

_cache = {}


def _build_nc():
    import concourse.mybir as mybir
    from concourse import bacc
    from concourse.tile import TileContext

    f32 = mybir.dt.float32
    f32r = mybir.dt.float32r
    bf16 = mybir.dt.bfloat16
    fp8 = mybir.dt.float8e4
    Exp = mybir.ActivationFunctionType.Exp
    DR = mybir.MatmulPerfMode.DoubleRow

    nc = bacc.Bacc("TRN2", target_bir_lowering=False)

    xt = nc.dram_tensor("xt", [C, T], bf16, kind="ExternalInput")
    wqk = nc.dram_tensor("wqk", [C, 512], bf16, kind="ExternalInput")
    wv = nc.dram_tensor("wv", [C, OC], bf16, kind="ExternalInput")
    wos = nc.dram_tensor("wos", [OC, C], bf16, kind="ExternalInput")
    csd = nc.dram_tensor("cs", [128, 2, T], bf16, kind="ExternalInput")
    cons = nc.dram_tensor("cons", [128, 384], bf16, kind="ExternalInput")
    onesd = nc.dram_tensor("ones", [65, 64], f32r, kind="ExternalInput")
    y = nc.dram_tensor("y", [T, C], bf16, kind="ExternalOutput")

    xr = xt.rearrange("(a p) t -> p a t", p=128)
    wqkr = wqk.rearrange("(a p) m -> p a m", p=128)
    wvr = wv.rearrange("(a p) m -> p a m", p=128)
    wor = wos.rearrange("(a p) m -> p a m", p=128)
    yr = y.rearrange("(a p) (o t) -> p a o t", p=128, o=2)

    with TileContext(nc) as tc:
        with tc.tile_pool(name="wgt", bufs=1) as wgt, \
             tc.tile_pool(name="persist", bufs=1) as persist, \
             tc.tile_pool(name="xtp", bufs=3) as xtp, \
             tc.tile_pool(name="rope_t", bufs=4) as rope_t, \
             tc.tile_pool(name="pairp", bufs=2) as pairp, \
             tc.tile_pool(name="estp", bufs=5) as estp, \
             tc.tile_pool(name="nrm", bufs=4) as nrm, \
             tc.tile_pool(name="ysb", bufs=2) as ysb, \
             tc.tile_pool(name="qk_ps", bufs=1, space="PSUM") as qk_ps, \
             tc.tile_pool(name="st_ps", bufs=2, space="PSUM") as st_ps, \
             tc.tile_pool(name="ot_ps", bufs=1, space="PSUM") as ot_ps, \
             tc.tile_pool(name="scr_ps", bufs=1, space="PSUM") as scr_ps:

            w_qk1 = wgt.tile([128, NCH, 256], bf16, tag="wqk1")
            w_qk2 = wgt.tile([128, NCH, 256], bf16, tag="wqk2")
            w_v = wgt.tile([128, NCH, OC], bf16, tag="wv")
            w_o = wgt.tile([128, 2, C], bf16, tag="wo")
            t_cs = wgt.tile([128, 2, T], bf16, tag="cs")
            t_cons = wgt.tile([128, 384], bf16, tag="cons")
            t_one = wgt.tile([65, 64], f32r, tag="one")
            bias_t = wgt.tile([128, 1], f32, tag="bias")
            tri2 = t_cons[:, 0:256]
            t_id = t_cons[:, 256:384]

            # rQ/rK: head h at partitions 64*(h%2):+64 of slot h//2; within a
            # head, dims = [32 evens | 32 odds] (repacked from rope layout)
            rQ = persist.tile([128, 2, T], bf16, tag="rq")
            rK = persist.tile([128, 2, T], bf16, tag="rk")
            Vt = persist.tile([128, NT * 2, HPC, 2, D + 1], bf16, tag="v")
            OCt = persist.tile([128, 2, T], bf16, tag="oc")

            nc.vector.memset(bias_t, -LN4)
            ones_sb = wgt.tile([128, NT * 2 * HPC * 2], bf16, tag="ones_sb")
            nc.vector.memset(ones_sb, 1.0)
            with nc.allow_low_precision(reason="softmax ones column"):
                nc.vector.tensor_copy(
                    out=Vt[:, :, :, :, D:D + 1],
                    in_=ones_sb.rearrange("p (a b c) -> p a b c", a=NT * 2, b=HPC))

            # ------- initial loads (sync + vector queues, HWDGE parallel) ---
            xts_of = {}
            xt0 = [xtp.tile([128, 2, 512], bf16, tag=f"xt0{i}",
                            name=f"xt0_{i}") for i in range(4)]
            xts_of[0] = ("split4", xt0)
            nc.gpsimd.dma_start(out=w_qk1[:, :, 0:128], in_=wqkr[:, :, 0:128])
            nc.sync.dma_start(out=xt0[0][:, 0:1, :], in_=xr[:, 0:1, 0:512])
            nc.gpsimd.dma_start(out=w_qk1[:, :, 128:256], in_=wqkr[:, :, 128:256])
            nc.sync.dma_start(out=xt0[0][:, 1:2, :], in_=xr[:, 1:2, 0:512])
            nc.sync.dma_start(out=xt0[1], in_=xr[:, 2:4, 0:512])
            nc.scalar.dma_start(out=xt0[2], in_=xr[:, 4:6, 0:512])
            nc.sync.dma_start(out=xt0[3], in_=xr[:, 6:8, 0:512])
            nc.scalar.dma_start(out=w_qk2, in_=wqkr[:, :, 256:512])
            nc.sync.dma_start(out=t_cs, in_=csd[:, :, :])
            nc.scalar.dma_start(out=w_v, in_=wvr[:, :, :])
            nc.sync.dma_start(out=t_cons, in_=cons[:, :])
            nc.scalar.dma_start(out=t_one, in_=onesd[:, :])
            nc.sync.dma_start(out=w_o, in_=wor[:, :, :])

            def xslice(tb, ci):
                e = xts_of[tb]
                if isinstance(e, tuple):
                    return e[1][ci // 2][:, ci % 2, :]
                return e[:, ci, :]

            def load_xt(tb):
                xtile = xtp.tile([128, NCH, 512], bf16, tag="xt", name=f"xt_{tb}")
                nc.sync.dma_start(out=xtile,
                                  in_=xr[:, :, tb * 512:tb * 512 + 512])
                xts_of[tb] = xtile

            # ---------------- builder pieces --------------------------------
            def proj_steps(tb, prefetch_first=True):
                """Closures for t-block tb's projections + rope + V."""
                ts = slice(tb * 512, tb * 512 + 512)
                steps = []
                state = {}

                def qk_mm(nmi, ci, nm=None):
                    if ci == 0:
                        state[nmi] = qk_ps.tile([128, 2, 512], f32, tag="qk",
                                                name=f"qk_{tb}_{nmi}")
                    p = state[nmi]
                    wt = w_qk1 if nmi == 0 else w_qk2
                    for eo in range(2):
                        nc.tensor.matmul(
                            p[:, eo, :], wt[:, ci, 128 * eo:128 * eo + 128],
                            xslice(tb, ci),
                            start=(ci == 0), stop=(ci == NCH - 1))

                def rope_muls(nmi):
                    p = state[nmi]
                    t1 = rope_t.tile([128, 512], f32, tag="t1")
                    t2 = rope_t.tile([128, 512], f32, tag="t2")
                    t3 = rope_t.tile([128, 512], f32, tag="t3")
                    t4 = rope_t.tile([128, 512], f32, tag="t4")
                    nc.vector.tensor_mul(t1, p[:, 0, :], t_cs[:, 0, ts])
                    nc.vector.tensor_mul(t2, p[:, 1, :], t_cs[:, 1, ts])
                    nc.vector.tensor_mul(t3, p[:, 1, :], t_cs[:, 0, ts])
                    nc.vector.tensor_mul(t4, p[:, 0, :], t_cs[:, 1, ts])
                    state[(nmi, "t")] = (t1, t2, t3, t4)

                def rope_addsub(nmi):
                    t1, t2, t3, t4 = state.pop((nmi, "t"))
                    pr = pairp.tile([128, 2, 512], bf16, tag="pair",
                                    name=f"pair_{tb}_{nmi}")
                    with nc.allow_low_precision(reason="bf16 attention operands"):
                        nc.vector.tensor_sub(pr[:, 0, :], t1, t2)
                        nc.vector.tensor_add(pr[:, 1, :], t3, t4)
                    state[(nmi, "pair")] = pr

                def repack(nmi, a):
                    # head a's evens/odds (rope layout, partitions 32a:+32) ->
                    # rQ/rK head layout: partitions 64*(a%2)+32*eo, slot a//2
                    pr = state[(nmi, "pair")]
                    dst = rQ if nmi == 0 else rK
                    q = nc.sync if nmi == 0 else nc.gpsimd
                    for eo in range(2):
                        q.dma_start(
                            out=dst[64 * (a % 2) + 32 * eo:
                                    64 * (a % 2) + 32 * eo + 32, a // 2, ts],
                            in_=pr[32 * a:32 * a + 32, eo, :])

                def v_mm(s):
                    pv = scr_ps.tile([128, 512], f32, tag="scr",
                                     name=f"pv_{tb}_{s}")
                    for ci in range(NCH):
                        nc.tensor.matmul(
                            pv[:, 0:OC], xslice(tb, ci)[:, 128 * s:128 * s + 128],
                            w_v[:, ci, :], start=(ci == 0), stop=(ci == NCH - 1))
                    state[("pv", s)] = pv

                def v_copy(s):
                    pv = state.pop(("pv", s))
                    pr, j = 2 * tb + s // 2, s % 2
                    with nc.allow_low_precision(reason="bf16 V"):
                        nc.scalar.copy(
                            out=Vt[:, pr, :, j, 0:D],
                            in_=pv[:, 0:OC].rearrange("p (h d) -> p h d", h=HPC))

                qk_steps = []
                if prefetch_first and tb + 1 < NT:
                    qk_steps.append(lambda: load_xt(tb + 1))
                for ci in range(NCH):
                    qk_steps.append(lambda ci=ci: qk_mm(0, ci))
                qk_steps.append(lambda: rope_muls(0))
                qk_steps.append(lambda: rope_addsub(0))
                for a in range(HPC):
                    qk_steps.append(lambda a=a: repack(0, a))
                for ci in range(NCH):
                    qk_steps.append(lambda ci=ci: qk_mm(1, ci))
                qk_steps.append(lambda: rope_muls(1))
                qk_steps.append(lambda: rope_addsub(1))
                for a in range(HPC):
                    qk_steps.append(lambda a=a: repack(1, a))
                v_steps = []
                for s in range(4):
                    v_steps.append(lambda s=s: v_mm(s))
                    v_steps.append(lambda s=s: v_copy(s))
                if not prefetch_first and tb + 1 < NT:
                    v_steps.append(lambda: load_xt(tb + 1))
                return qk_steps, v_steps

            def yproj_steps(w, pools=None, fine=False):
                """Closures for output projection of window w."""
                steps = []
                state = {"n": 0}
                pools = pools or [(scr_ps, [128, 512], "scr")]

                def py_mm(tsub, ob2):
                    if "sy" not in state:
                        state["sy"] = ysb.tile([128, 4, 2, 512], bf16, tag="sy",
                                               name=f"sy_{w}")
                    tsl = slice((4 * w + tsub) * 128, (4 * w + tsub) * 128 + 128)
                    pool, shape, tag = pools[state["n"] % len(pools)]
                    state["n"] += 1
                    py = pool.tile(shape, f32, tag=tag,
                                   name=f"py_{w}_{tsub}_{ob2}")
                    if len(shape) == 3:
                        py = py[:, 0, :]
                    for k in range(2):
                        nc.tensor.matmul(
                            py, OCt[:, k, tsl], w_o[:, k, 512 * ob2:512 * ob2 + 512],
                            start=(k == 0), stop=(k == 1))
                    with nc.allow_low_precision(reason="bf16 output"):
                        if w < 2 or (fine and (tsub + ob2) % 2 == 1):
                            nc.scalar.copy(out=state["sy"][:, tsub, ob2, :],
                                           in_=py)
                        else:
                            nc.vector.tensor_copy(
                                out=state["sy"][:, tsub, ob2, :], in_=py)

                def y_dma(tsub, ob2=None):
                    if ob2 is None:
                        nc.sync.dma_start(out=yr[:, 4 * w + tsub, :, :],
                                          in_=state["sy"][:, tsub, :, :])
                    else:
                        nc.sync.dma_start(out=yr[:, 4 * w + tsub, ob2, :],
                                          in_=state["sy"][:, tsub, ob2, :])

                for tsub in range(4):
                    for ob2 in range(2):
                        steps.append(lambda tsub=tsub, ob2=ob2: py_mm(tsub, ob2))
                        if fine:
                            steps.append(
                                lambda tsub=tsub, ob2=ob2: y_dma(tsub, ob2))
                    if not fine:
                        steps.append(lambda tsub=tsub: y_dma(tsub))
                return steps

            # ---------------- main schedule ---------------------------------
            qk0, v0 = proj_steps(0, prefetch_first=False)
            for step in qk0 + v0:
                step()

            for w in range(NT):
                chain = []
                fillers = []
                if w + 1 < NT:
                    qkn, vn = proj_steps(w + 1)
                    chain += qkn
                    fillers += vn
                if w > 0:
                    if w == NT - 1:
                        fillers += yproj_steps(w - 1, pools=[
                            (scr_ps, [128, 512], "scr"),
                            (qk_ps, [128, 2, 512], "qk")])
                    else:
                        fillers += yproj_steps(w - 1)
                qlo = 512 * w
                npair = 2 * w + 2
                slots = HPC * npair

                def pop_fillers(frac_done):
                    # qk chain front-loaded hard; bulk fillers gentler
                    want_c = int(len_c0 * min(1.0, 2.2 * frac_done) + 0.999)
                    while emitted_c[0] < want_c and chain:
                        chain.pop(0)()
                        emitted_c[0] += 1
                    want = int(len_f0 * min(1.0, 1.5 * frac_done) + 0.999)
                    while emitted[0] < want and fillers:
                        fillers.pop(0)()
                        emitted[0] += 1

                len_c0 = len(chain)
                len_f0 = len(fillers)
                emitted_c = [0]
                emitted = [0]
                slot = [0]

                for h in (1, 3, 0, 2):
                    hb, ob = 64 * (h % 2), h // 2
                    ot = ot_ps.tile([128, 512], f32, tag="ot", name=f"ot_{w}_{h}")
                    pend = []

                    def emit_pv(h=h, w=w, ot=ot):
                        # one matmul per 128-key chunk
                        p, est, lo = pend.pop(0)
                        for j in range(2):
                            nc.tensor.matmul(
                                ot[0:D + 1, lo:512], Vt[:, p, h, j, :],
                                est[:, j, lo:512],
                                start=(p == 0 and j == 0),
                                stop=(p == npair - 1 and j == 1),
                                skip_group_check=True)

                    for p in range(npair):
                        lo = 256 if p == npair - 1 else 0
                        st = st_ps.tile([128, 2, 512], f32, tag="st")
                        for j, c in enumerate((2 * p, 2 * p + 1)):
                            rk = 128 * c - qlo
                            diag = rk >= 0
                            nc.tensor.matmul(
                                st[:, j, lo:512],
                                rK[hb:hb + 64, ob, 128 * c:128 * c + 128],
                                rQ[hb:hb + 64, ob, qlo + lo:qlo + 512],
                                start=True, stop=not diag)
                            if diag:
                                ms = max(lo, rk - 128)
                                nc.tensor.matmul(
                                    st[:, j, ms:rk + 128], t_id,
                                    tri2[:, 128 - (rk - ms):256],
                                    start=False, stop=True,
                                    skip_group_check=True)
                        est = estp.tile([128, 2, 512], bf16, tag="est")
                        with nc.allow_low_precision(reason="bf16 softmax"):
                            nc.scalar.activation(out=est[:, :, lo:512],
                                                 in_=st[:, :, lo:512],
                                                 func=Exp, scale=0.125,
                                                 bias=bias_t[:, :])
                        pend.append((p, est, lo))
                        if len(pend) > 3:
                            emit_pv()
                        slot[0] += 1
                        pop_fillers(slot[0] / slots)
                    while pend:
                        emit_pv()
                    # normalize: 1/l broadcast across D partitions, scale, store
                    rl = nrm.tile([65, 512], f32r, tag="rl")
                    with nc.allow_low_precision(reason="1/l feeds matmul"):
                        nc.vector.reciprocal(out=rl[64:65, :], in_=ot[64:65, :])
                    rlb = scr_ps.tile([128, 512], f32, tag="scr",
                                      name=f"rlb_{w}_{h}")
                    nc.tensor.matmul(rlb[0:D, :], t_one[64:65, :], rl[64:65, :],
                                     start=True, stop=True)
                    rlb_sb = nrm.tile([64, 512], f32, tag="rlbsb")
                    nc.gpsimd.tensor_copy(out=rlb_sb, in_=rlb[0:D, :])
                    with nc.allow_low_precision(reason="bf16 attention out"):
                        if h % 2 == 0:
                            # partition-aligned: write OCt rows 0:64 directly
                            nc.vector.tensor_mul(
                                OCt[0:D, ob, qlo:qlo + 512], ot[0:D, :], rlb_sb)
                        else:
                            otn = nrm.tile([64, 512], bf16, tag="otn")
                            nc.vector.tensor_mul(otn, ot[0:D, :], rlb_sb)
                            nc.sync.dma_start(
                                out=OCt[64:64 + D, ob, qlo:qlo + 512], in_=otn)
                while chain:
                    chain.pop(0)()
                while fillers:
                    fillers.pop(0)()

            for step in yproj_steps(NT - 1, pools=[
                    (scr_ps, [128, 512], "scr"),
                    (st_ps, [128, 2, 512], "st"),
                    (qk_ps, [128, 2, 512], "qk")], fine=True):
                step()
    nc.compile()
    return nc


def _prep_inputs(x, wq, wk, wv, wo, rope_cos, rope_sin):
    """Host-side sharding/pre-transposition. Core i: batch i//4, head group i%4."""
    import ml_dtypes
    f = np.float32
    bf = ml_dtypes.bfloat16
    COS = np.tile(np.ascontiguousarray(rope_cos.T.astype(f)), (4, 1))  # [128,T]
    SIN = np.tile(np.ascontiguousarray(rope_sin.T.astype(f)), (4, 1))
    cs = np.stack([COS, SIN], axis=1).astype(bf)                       # [128,2,T]
    cons = np.zeros((128, 384), f)
    cons[:, 0:128] = NEG
    cons[:, 128:256] = np.where(
        np.arange(128)[:, None] > np.arange(128)[None, :], f(NEG), f(0.0))
    cons[:, 256:384] = np.eye(128, dtype=f)
    cons = cons.astype(bf)
    xT = [np.ascontiguousarray(x[b].T.astype(bf)) for b in range(B)]
    in_maps = []
    for core in range(8):
        b, g = core // 4, core % 4
        heads = [4 * g + a for a in range(HPC)]
        e_rows = np.concatenate([64 * h + 2 * np.arange(32) for h in heads])
        o_rows = e_rows + 1
        sl = slice(OC * g, OC * g + OC)
        wqkc = np.concatenate(
            [wq[e_rows].T, wq[o_rows].T, wk[e_rows].T, wk[o_rows].T],
            axis=1).astype(bf)
        in_maps.append({
            "xt": xT[b],
            "wqk": np.ascontiguousarray(wqkc),
            "wv": np.ascontiguousarray(wv[sl].T.astype(bf)),
            "wos": np.ascontiguousarray(wo[:, sl].T.astype(bf)),
            "cs": cs, "cons": cons,
            "ones": np.ones((65, 64), np.float32),
        })
    return in_maps


def kernel(x, wq, wk, wv, wo, rope_cos, rope_sin, _trace=False):
    from concourse.bass_utils import run_bass_kernel_spmd
    if "nc" not in _cache:
        _cache["nc"] = _build_nc()
    nc = _cache["nc"]
    in_maps = _prep_inputs(np.asarray(x), np.asarray(wq), np.asarray(wk),
                           np.asarray(wv), np.asarray(wo),
                           np.asarray(rope_cos), np.asarray(rope_sin))
    res = run_bass_kernel_spmd(nc, in_maps, core_ids=list(range(8)),
                               trace=_trace)
    _cache["last_result"] = res
    out = np.zeros((B, T, C), np.float32)
    for core in range(8):
        out[core // 4] += res.results[core]["y"]
    return out


# revision 48
# speedup vs baseline: 1.0243x; 1.0243x over previous
import sys
sys.path.insert(0, "/opt/trn_rl_repo")
import numpy as np

B, T, C, H = 2, 2048, 1024, 16
D = C // H          # 64
HPC = 4             # heads per core
OC = HPC * D        # 256 v-channels per core
NT = T // 512       # 4 query windows / t-blocks
NCH = C // 128      # 8 contraction chunks
NEG = -1e30
LN4 = 1.3862943611198906  # exp bias: est = exp(s/8)/4 keeps fp8e4 range safe

_cache = {}


def _build_nc():
    import concourse.mybir as mybir
    from concourse import bacc
    from concourse.tile import TileContext

    f32 = mybir.dt.float32
    f32r = mybir.dt.float32r
    bf16 = mybir.dt.bfloat16
    fp8 = mybir.dt.float8e4
    Exp = mybir.ActivationFunctionType.Exp
    DR = mybir.MatmulPerfMode.DoubleRow

    nc = bacc.Bacc("TRN2", target_bir_lowering=False)

    xt = nc.dram_tensor("xt", [C, T], bf16, kind="ExternalInput")
    wqk = nc.dram_tensor("wqk", [C, 512], bf16, kind="ExternalInput")
    wv = nc.dram_tensor("wv", [C, OC], bf16, kind="ExternalInput")
    wos = nc.dram_tensor("wos", [OC, C], bf16, kind="ExternalInput")
    csd = nc.dram_tensor("cs", [128, 2, T], bf16, kind="ExternalInput")
    cons = nc.dram_tensor("cons", [128, 384], bf16, kind="ExternalInput")
    onesd = nc.dram_tensor("ones", [65, 64], f32r, kind="ExternalInput")
    y = nc.dram_tensor("y", [T, C], bf16, kind="ExternalOutput")

    xr = xt.rearrange("(a p) t -> p a t", p=128)
    wqkr = wqk.rearrange("(a p) m -> p a m", p=128)
    wvr = wv.rearrange("(a p) m -> p a m", p=128)
    wor = wos.rearrange("(a p) m -> p a m", p=128)
    yr = y.rearrange("(a p) (o t) -> p a o t", p=128, o=2)

    with TileContext(nc) as tc:
        with tc.tile_pool(name="wgt", bufs=1) as wgt, \
             tc.tile_pool(name="persist", bufs=1) as persist, \
             tc.tile_pool(name="xtp", bufs=3) as xtp, \
             tc.tile_pool(name="rope_t", bufs=4) as rope_t, \
             tc.tile_pool(name="pairp", bufs=2) as pairp, \
             tc.tile_pool(name="estp", bufs=5) as estp, \
             tc.tile_pool(name="nrm", bufs=4) as nrm, \
             tc.tile_pool(name="ysb", bufs=2) as ysb, \
             tc.tile_pool(name="qk_ps", bufs=1, space="PSUM") as qk_ps, \
             tc.tile_pool(name="st_ps", bufs=2, space="PSUM") as st_ps, \
             tc.tile_pool(name="ot_ps", bufs=1, space="PSUM") as ot_ps, \
             tc.tile_pool(name="scr_ps", bufs=1, space="PSUM") as scr_ps:

            w_qk1 = wgt.tile([128, NCH, 256], bf16, tag="wqk1")
            w_qk2 = wgt.tile([128, NCH, 256], bf16, tag="wqk2")
            w_v = wgt.tile([128, NCH, OC], bf16, tag="wv")
            w_o = wgt.tile([128, 2, C], bf16, tag="wo")
            t_cs = wgt.tile([128, 2, T], bf16, tag="cs")
            t_cons = wgt.tile([128, 384], bf16, tag="cons")
            t_one = wgt.tile([65, 64], f32r, tag="one")
            bias_t = wgt.tile([128, 1], f32, tag="bias")
            tri2 = t_cons[:, 0:256]
            t_id = t_cons[:, 256:384]

            # rQ/rK: head h at partitions 64*(h%2):+64 of slot h//2; within a
            # head, dims = [32 evens | 32 odds] (repacked from rope layout)
            rQ = persist.tile([128, 2, T], bf16, tag="rq")
            rK = persist.tile([128, 2, T], bf16, tag="rk")
            Vt = persist.tile([128, NT * 2, HPC, 2, D + 1], bf16, tag="v")
            OCt = persist.tile([128, 2, T], bf16, tag="oc")

            nc.vector.memset(bias_t, -LN4)
            ones_sb = wgt.tile([128, NT * 2 * HPC * 2], bf16, tag="ones_sb")
            nc.vector.memset(ones_sb, 1.0)
            with nc.allow_low_precision(reason="softmax ones column"):
                nc.vector.tensor_copy(
                    out=Vt[:, :, :, :, D:D + 1],
                    in_=ones_sb.rearrange("p (a b c) -> p a b c", a=NT * 2, b=HPC))

            # ------- initial loads (sync + vector queues, HWDGE parallel) ---
            xts_of = {}
            xt0 = [xtp.tile([128, 2, 512], bf16, tag=f"xt0{i}",
                            name=f"xt0_{i}") for i in range(4)]
            xts_of[0] = ("split4", xt0)
            nc.gpsimd.dma_start(out=w_qk1[:, :, 0:128], in_=wqkr[:, :, 0:128])
            nc.sync.dma_start(out=xt0[0][:, 0:1, :], in_=xr[:, 0:1, 0:512])
            nc.scalar.dma_start(out=w_qk1[:, :, 128:256], in_=wqkr[:, :, 128:256])
            nc.sync.dma_start(out=xt0[0][:, 1:2, :], in_=xr[:, 1:2, 0:512])
            nc.sync.dma_start(out=xt0[1], in_=xr[:, 2:4, 0:512])
            nc.scalar.dma_start(out=xt0[2], in_=xr[:, 4:6, 0:512])
            nc.sync.dma_start(out=xt0[3], in_=xr[:, 6:8, 0:512])
            nc.scalar.dma_start(out=w_qk2, in_=wqkr[:, :, 256:512])
            nc.sync.dma_start(out=t_cs, in_=csd[:, :, :])
            nc.scalar.dma_start(out=w_v, in_=wvr[:, :, :])
            nc.sync.dma_start(out=t_cons, in_=cons[:, :])
            nc.scalar.dma_start(out=t_one, in_=onesd[:, :])
            nc.sync.dma_start(out=w_o, in_=wor[:, :, :])

            def xslice(tb, ci):
                e = xts_of[tb]
                if isinstance(e, tuple):
                    return e[1][ci // 2][:, ci % 2, :]
                return e[:, ci, :]

            def load_xt(tb):
                xtile = xtp.tile([128, NCH, 512], bf16, tag="xt", name=f"xt_{tb}")
                nc.sync.dma_start(out=xtile[:, 0:4, :],
                                  in_=xr[:, 0:4, tb * 512:tb * 512 + 512])
                nc.sync.dma_start(out=xtile[:, 4:8, :],
                                  in_=xr[:, 4:8, tb * 512:tb * 512 + 512])
                xts_of[tb] = xtile

            # ---------------- builder pieces --------------------------------
            def proj_steps(tb, prefetch_first=True):
                """Closures for t-block tb's projections + rope + V."""
                ts = slice(tb * 512, tb * 512 + 512)
                steps = []
                state = {}

                def qk_mm(nmi, ci, nm=None):
                    if ci == 0:
                        state[nmi] = qk_ps.tile([128, 2, 512], f32, tag="qk",
                                                name=f"qk_{tb}_{nmi}")
                    p = state[nmi]
                    wt = w_qk1 if nmi == 0 else w_qk2
                    for eo in range(2):
                        nc.tensor.matmul(
                            p[:, eo, :], wt[:, ci, 128 * eo:128 * eo + 128],
                            xslice(tb, ci),
                            start=(ci == 0), stop=(ci == NCH - 1))

                def rope_muls(nmi):
                    p = state[nmi]
                    t1 = rope_t.tile([128, 512], f32, tag="t1")
                    t2 = rope_t.tile([128, 512], f32, tag="t2")
                    t3 = rope_t.tile([128, 512], f32, tag="t3")
                    t4 = rope_t.tile([128, 512], f32, tag="t4")
                    nc.vector.tensor_mul(t1, p[:, 0, :], t_cs[:, 0, ts])
                    nc.vector.tensor_mul(t2, p[:, 1, :], t_cs[:, 1, ts])
                    nc.vector.tensor_mul(t3, p[:, 1, :], t_cs[:, 0, ts])
                    nc.vector.tensor_mul(t4, p[:, 0, :], t_cs[:, 1, ts])
                    state[(nmi, "t")] = (t1, t2, t3, t4)

                def rope_addsub(nmi):
                    t1, t2, t3, t4 = state.pop((nmi, "t"))
                    pr = pairp.tile([128, 2, 512], bf16, tag="pair",
                                    name=f"pair_{tb}_{nmi}")
                    with nc.allow_low_precision(reason="bf16 attention operands"):
                        nc.vector.tensor_sub(pr[:, 0, :], t1, t2)
                        nc.vector.tensor_add(pr[:, 1, :], t3, t4)
                    state[(nmi, "pair")] = pr

                def repack(nmi, a):
                    # head a's evens/odds (rope layout, partitions 32a:+32) ->
                    # rQ/rK head layout: partitions 64*(a%2)+32*eo, slot a//2
                    pr = state[(nmi, "pair")]
                    dst = rQ if nmi == 0 else rK
                    q = nc.sync if nmi == 0 else nc.gpsimd
                    for eo in range(2):
                        q.dma_start(
                            out=dst[64 * (a % 2) + 32 * eo:
                                    64 * (a % 2) + 32 * eo + 32, a // 2, ts],
                            in_=pr[32 * a:32 * a + 32, eo, :])

                def v_mm(s):
                    pv = scr_ps.tile([128, 512], f32, tag="scr",
                                     name=f"pv_{tb}_{s}")
                    for ci in range(NCH):
                        nc.tensor.matmul(
                            pv[:, 0:OC], xslice(tb, ci)[:, 128 * s:128 * s + 128],
                            w_v[:, ci, :], start=(ci == 0), stop=(ci == NCH - 1))
                    state[("pv", s)] = pv

                def v_copy(s):
                    pv = state.pop(("pv", s))
                    pr, j = 2 * tb + s // 2, s % 2
                    with nc.allow_low_precision(reason="bf16 V"):
                        nc.scalar.copy(
                            out=Vt[:, pr, :, j, 0:D],
                            in_=pv[:, 0:OC].rearrange("p (h d) -> p h d", h=HPC))

                qk_steps = []
                if prefetch_first and tb + 1 < NT:
                    qk_steps.append(lambda: load_xt(tb + 1))
                for ci in range(NCH):
                    qk_steps.append(lambda ci=ci: qk_mm(0, ci))
                qk_steps.append(lambda: rope_muls(0))
                qk_steps.append(lambda: rope_addsub(0))
                for a in range(HPC):
                    qk_steps.append(lambda a=a: repack(0, a))
                for ci in range(NCH):
                    qk_steps.append(lambda ci=ci: qk_mm(1, ci))
                qk_steps.append(lambda: rope_muls(1))
                qk_steps.append(lambda: rope_addsub(1))
                for a in range(HPC):
                    qk_steps.append(lambda a=a: repack(1, a))
                v_steps = []
                for s in range(4):
                    v_steps.append(lambda s=s: v_mm(s))
                    v_steps.append(lambda s=s: v_copy(s))
                if not prefetch_first and tb + 1 < NT:
                    v_steps.append(lambda: load_xt(tb + 1))
                return qk_steps, v_steps

            def yproj_steps(w, pools=None, fine=False):
                """Closures for output projection of window w."""
                steps = []
                state = {"n": 0}
                pools = pools or [(scr_ps, [128, 512], "scr")]

                def py_mm(tsub, ob2):
                    if "sy" not in state:
                        state["sy"] = ysb.tile([128, 4, 2, 512], bf16, tag="sy",
                                               name=f"sy_{w}")
                    tsl = slice((4 * w + tsub) * 128, (4 * w + tsub) * 128 + 128)
                    pool, shape, tag = pools[state["n"] % len(pools)]
                    state["n"] += 1
                    py = pool.tile(shape, f32, tag=tag,
                                   name=f"py_{w}_{tsub}_{ob2}")
                    if len(shape) == 3:
                        py = py[:, 0, :]
                    for k in range(2):
                        nc.tensor.matmul(
                            py, OCt[:, k, tsl], w_o[:, k, 512 * ob2:512 * ob2 + 512],
                            start=(k == 0), stop=(k == 1))
                    with nc.allow_low_precision(reason="bf16 output"):
                        if w < 2 or (fine and (tsub + ob2) % 2 == 1):
                            nc.scalar.copy(out=state["sy"][:, tsub, ob2, :],
                                           in_=py)
                        else:
                            nc.vector.tensor_copy(
                                out=state["sy"][:, tsub, ob2, :], in_=py)

                def y_dma(tsub, ob2=None):
                    if ob2 is None:
                        nc.sync.dma_start(out=yr[:, 4 * w + tsub, :, :],
                                          in_=state["sy"][:, tsub, :, :])
                    else:
                        nc.sync.dma_start(out=yr[:, 4 * w + tsub, ob2, :],
                                          in_=state["sy"][:, tsub, ob2, :])

                for tsub in range(4):
                    for ob2 in range(2):
                        steps.append(lambda tsub=tsub, ob2=ob2: py_mm(tsub, ob2))
                        if fine:
                            steps.append(
                                lambda tsub=tsub, ob2=ob2: y_dma(tsub, ob2))
                    if not fine:
                        steps.append(lambda tsub=tsub: y_dma(tsub))
                return steps

            # ---------------- main schedule ---------------------------------
            qk0, v0 = proj_steps(0, prefetch_first=False)
            for step in qk0 + v0:
                step()

            for w in range(NT):
                chain = []
                fillers = []
                if w + 1 < NT:
                    qkn, vn = proj_steps(w + 1)
                    chain += qkn
                    fillers += vn
                if w > 0:
                    if w == NT - 1:
                        fillers += yproj_steps(w - 1, pools=[
                            (scr_ps, [128, 512], "scr"),
                            (qk_ps, [128, 2, 512], "qk")])
                    else:
                        fillers += yproj_steps(w - 1)
                qlo = 512 * w
                npair = 2 * w + 2
                slots = HPC * npair

                def pop_fillers(frac_done):
                    # qk chain front-loaded hard; bulk fillers gentler
                    want_c = int(len_c0 * min(1.0, 2.2 * frac_done) + 0.999)
                    while emitted_c[0] < want_c and chain:
                        chain.pop(0)()
                        emitted_c[0] += 1
                    want = int(len_f0 * min(1.0, 1.5 * frac_done) + 0.999)
                    while emitted[0] < want and fillers:
                        fillers.pop(0)()
                        emitted[0] += 1

                len_c0 = len(chain)
                len_f0 = len(fillers)
                emitted_c = [0]
                emitted = [0]
                slot = [0]

                for h in (1, 3, 0, 2):
                    hb, ob = 64 * (h % 2), h // 2
                    ot = ot_ps.tile([128, 512], f32, tag="ot", name=f"ot_{w}_{h}")
                    pend = []

                    def emit_pv(h=h, w=w, ot=ot):
                        # one matmul per 128-key chunk
                        p, est, lo = pend.pop(0)
                        for j in range(2):
                            nc.tensor.matmul(
                                ot[0:D + 1, lo:512], Vt[:, p, h, j, :],
                                est[:, j, lo:512],
                                start=(p == 0 and j == 0),
                                stop=(p == npair - 1 and j == 1),
                                skip_group_check=True)

                    for p in range(npair):
                        lo = 256 if p == npair - 1 else 0
                        st = st_ps.tile([128, 2, 512], f32, tag="st")
                        for j, c in enumerate((2 * p, 2 * p + 1)):
                            rk = 128 * c - qlo
                            diag = rk >= 0
                            nc.tensor.matmul(
                                st[:, j, lo:512],
                                rK[hb:hb + 64, ob, 128 * c:128 * c + 128],
                                rQ[hb:hb + 64, ob, qlo + lo:qlo + 512],
                                start=True, stop=not diag)
                            if diag:
                                ms = max(lo, rk - 128)
                                nc.tensor.matmul(
                                    st[:, j, ms:rk + 128], t_id,
                                    tri2[:, 128 - (rk - ms):256],
                                    start=False, stop=True,
                                    skip_group_check=True)
                        est = estp.tile([128, 2, 512], bf16, tag="est")
                        with nc.allow_low_precision(reason="bf16 softmax"):
                            nc.scalar.activation(out=est[:, :, lo:512],
                                                 in_=st[:, :, lo:512],
                                                 func=Exp, scale=0.125,
                                                 bias=bias_t[:, :])
                        pend.append((p, est, lo))
                        if len(pend) > 3:
                            emit_pv()
                        slot[0] += 1
                        pop_fillers(slot[0] / slots)
                    while pend:
                        emit_pv()
                    # normalize: 1/l broadcast across D partitions, scale, store
                    rl = nrm.tile([65, 512], f32r, tag="rl")
                    with nc.allow_low_precision(reason="1/l feeds matmul"):
                        nc.vector.reciprocal(out=rl[64:65, :], in_=ot[64:65, :])
                    rlb = scr_ps.tile([128, 512], f32, tag="scr",
                                      name=f"rlb_{w}_{h}")
                    nc.tensor.matmul(rlb[0:D, :], t_one[64:65, :], rl[64:65, :],
                                     start=True, stop=True)
                    rlb_sb = nrm.tile([64, 512], f32, tag="rlbsb")
                    nc.gpsimd.tensor_copy(out=rlb_sb, in_=rlb[0:D, :])
                    with nc.allow_low_precision(reason="bf16 attention out"):
                        if h % 2 == 0:
                            # partition-aligned: write OCt rows 0:64 directly
                            nc.vector.tensor_mul(
                                OCt[0:D, ob, qlo:qlo + 512], ot[0:D, :], rlb_sb)
                        else:
                            otn = nrm.tile([64, 512], bf16, tag="otn")
                            nc.vector.tensor_mul(otn, ot[0:D, :], rlb_sb)
                            nc.sync.dma_start(
                                out=OCt[64:64 + D, ob, qlo:qlo + 512], in_=otn)
                while chain:
                    chain.pop(0)()
                while fillers:
                    fillers.pop(0)()

            for step in yproj_steps(NT - 1, pools=[
                    (scr_ps, [128, 512], "scr"),
                    (st_ps, [128, 2, 512], "st"),
                    (qk_ps, [128, 2, 512], "qk")], fine=True):
                step()
    nc.compile()
    return nc


def _prep_inputs(x, wq, wk, wv, wo, rope_cos, rope_sin):
    """Host-side sharding/pre-transposition. Core i: batch i//4, head group i%4."""
    import ml_dtypes
    f = np.float32
    bf = ml_dtypes.bfloat16
    COS = np.tile(np.ascontiguousarray(rope_cos.T.astype(f)), (4, 1))  # [128,T]
    SIN = np.tile(np.ascontiguousarray(rope_sin.T.astype(f)), (4, 1))
    cs = np.stack([COS, SIN], axis=1).astype(bf)                       # [128,2,T]
    cons = np.zeros((128, 384), f)
    cons[:, 0:128] = NEG
    cons[:, 128:256] = np.where(
        np.arange(128)[:, None] > np.arange(128)[None, :], f(NEG), f(0.0))
    cons[:, 256:384] = np.eye(128, dtype=f)
    cons = cons.astype(bf)
    xT = [np.ascontiguousarray(x[b].T.astype(bf)) for b in range(B)]
    in_maps = []
    for core in range(8):
        b, g = core // 4, core % 4
        heads = [4 * g + a for a in range(HPC)]
        e_rows = np.concatenate([64 * h + 2 * np.arange(32) for h in heads])
        o_rows = e_rows + 1
        sl = slice(OC * g, OC * g + OC)
        wqkc = np.concatenate(
            [wq[e_rows].T, wq[o_rows].T, wk[e_rows].T, wk[o_rows].T],
            axis=1).astype(bf)
        in_maps.append({
            "xt": xT[b],
            "wqk": np.ascontiguousarray(wqkc),
            "wv": np.ascontiguousarray(wv[sl].T.astype(bf)),
            "wos": np.ascontiguousarray(wo[:, sl].T.astype(bf)),
            "cs": cs, "cons": cons,
            "ones": np.ones((65, 64), np.float32),
        })
    return in_maps


def kernel(x, wq, wk, wv, wo, rope_cos, rope_sin, _trace=False):
    from concourse.bass_utils import run_bass_kernel_spmd
    if "nc" not in _cache:
        _cache["nc"] = _build_nc()
    nc = _cache["nc"]
    in_maps = _prep_inputs(np.asarray(x), np.asarray(wq), np.asarray(wk),
                           np.asarray(wv), np.asarray(wo),
                           np.asarray(rope_cos), np.asarray(rope_sin))
    res = run_bass_kernel_spmd(nc, in_maps, core_ids=list(range(8)),
                               trace=_trace)
    _cache["last_result"] = res
    out = np.zeros((B, T, C), np.float32)
    for core in range(8):
        out[core // 4] += res.results[core]["y"]
    return out


# revision 49
# speedup vs baseline: 1.0246x; 1.0003x over previous
import sys
sys.path.insert(0, "/opt/trn_rl_repo")
import numpy as np

B, T, C, H = 2, 2048, 1024, 16
D = C // H          # 64
HPC = 4             # heads per core
OC = HPC * D        # 256 v-channels per core
NT = T // 512       # 4 query windows / t-blocks
NCH = C // 128      # 8 contraction chunks
NEG = -1e30
LN4 = 1.3862943611198906  # exp bias: est = exp(s/8)/4 keeps fp8e4 range safe

_cache = {}


def _build_nc():
    import concourse.mybir as mybir
    from concourse import bacc
    from concourse.tile import TileContext

    f32 = mybir.dt.float32
    f32r = mybir.dt.float32r
    bf16 = mybir.dt.bfloat16
    fp8 = mybir.dt.float8e4
    Exp = mybir.ActivationFunctionType.Exp
    DR = mybir.MatmulPerfMode.DoubleRow

    nc = bacc.Bacc("TRN2", target_bir_lowering=False)

    xt = nc.dram_tensor("xt", [C, T], bf16, kind="ExternalInput")
    wqk = nc.dram_tensor("wqk", [C, 512], bf16, kind="ExternalInput")
    wv = nc.dram_tensor("wv", [C, OC], bf16, kind="ExternalInput")
    wos = nc.dram_tensor("wos", [OC, C], bf16, kind="ExternalInput")
    csd = nc.dram_tensor("cs", [128, 2, T], bf16, kind="ExternalInput")
    cons = nc.dram_tensor("cons", [128, 384], bf16, kind="ExternalInput")
    onesd = nc.dram_tensor("ones", [65, 64], f32r, kind="ExternalInput")
    y = nc.dram_tensor("y", [T, C], bf16, kind="ExternalOutput")

    xr = xt.rearrange("(a p) t -> p a t", p=128)
    wqkr = wqk.rearrange("(a p) m -> p a m", p=128)
    wvr = wv.rearrange("(a p) m -> p a m", p=128)
    wor = wos.rearrange("(a p) m -> p a m", p=128)
    yr = y.rearrange("(a p) (o t) -> p a o t", p=128, o=2)

    with TileContext(nc) as tc:
        with tc.tile_pool(name="wgt", bufs=1) as wgt, \
             tc.tile_pool(name="persist", bufs=1) as persist, \
             tc.tile_pool(name="xtp", bufs=3) as xtp, \
             tc.tile_pool(name="rope_t", bufs=4) as rope_t, \
             tc.tile_pool(name="pairp", bufs=2) as pairp, \
             tc.tile_pool(name="estp", bufs=5) as estp, \
             tc.tile_pool(name="nrm", bufs=4) as nrm, \
             tc.tile_pool(name="ysb", bufs=2) as ysb, \
             tc.tile_pool(name="qk_ps", bufs=1, space="PSUM") as qk_ps, \
             tc.tile_pool(name="st_ps", bufs=2, space="PSUM") as st_ps, \
             tc.tile_pool(name="ot_ps", bufs=1, space="PSUM") as ot_ps, \
             tc.tile_pool(name="scr_ps", bufs=1, space="PSUM") as scr_ps:

            w_qk1 = wgt.tile([128, NCH, 256], bf16, tag="wqk1")
            w_qk2 = wgt.tile([128, NCH, 256], bf16, tag="wqk2")
            w_v = wgt.tile([128, NCH, OC], bf16, tag="wv")
            w_o = wgt.tile([128, 2, C], bf16, tag="wo")
            t_cs = wgt.tile([128, 2, T], bf16, tag="cs")
            t_cons = wgt.tile([128, 384], bf16, tag="cons")
            t_one = wgt.tile([65, 64], f32r, tag="one")
            bias_t = wgt.tile([128, 1], f32, tag="bias")
            tri2 = t_cons[:, 0:256]
            t_id = t_cons[:, 256:384]

            # rQ/rK: head h at partitions 64*(h%2):+64 of slot h//2; within a
            # head, dims = [32 evens | 32 odds] (repacked from rope layout)
            rQ = persist.tile([128, 2, T], bf16, tag="rq")
            rK = persist.tile([128, 2, T], bf16, tag="rk")
            Vt = persist.tile([128, NT * 2, HPC, 2, D + 1], bf16, tag="v")
            OCt = persist.tile([128, 2, T], bf16, tag="oc")

            nc.vector.memset(bias_t, -LN4)
            ones_sb = wgt.tile([128, NT * 2 * HPC * 2], bf16, tag="ones_sb")
            nc.vector.memset(ones_sb, 1.0)
            with nc.allow_low_precision(reason="softmax ones column"):
                nc.vector.tensor_copy(
                    out=Vt[:, :, :, :, D:D + 1],
                    in_=ones_sb.rearrange("p (a b c) -> p a b c", a=NT * 2, b=HPC))

            # ------- initial loads (sync + vector queues, HWDGE parallel) ---
            xts_of = {}
            xt0 = [xtp.tile([128, 2, 512], bf16, tag=f"xt0{i}",
                            name=f"xt0_{i}") for i in range(4)]
            xts_of[0] = ("split4", xt0)
            nc.gpsimd.dma_start(out=w_qk1[:, :, 0:128], in_=wqkr[:, :, 0:128])
            nc.sync.dma_start(out=xt0[0][:, 0:1, :], in_=xr[:, 0:1, 0:512])
            nc.scalar.dma_start(out=w_qk1[:, :, 128:256], in_=wqkr[:, :, 128:256])
            nc.sync.dma_start(out=xt0[0][:, 1:2, :], in_=xr[:, 1:2, 0:512])
            nc.sync.dma_start(out=xt0[1], in_=xr[:, 2:4, 0:512])
            nc.scalar.dma_start(out=xt0[2], in_=xr[:, 4:6, 0:512])
            nc.sync.dma_start(out=xt0[3], in_=xr[:, 6:8, 0:512])
            nc.scalar.dma_start(out=w_qk2, in_=wqkr[:, :, 256:512])
            nc.sync.dma_start(out=t_cs, in_=csd[:, :, :])
            nc.scalar.dma_start(out=w_v, in_=wvr[:, :, :])
            nc.sync.dma_start(out=t_cons, in_=cons[:, :])
            nc.scalar.dma_start(out=t_one, in_=onesd[:, :])
            nc.sync.dma_start(out=w_o, in_=wor[:, :, :])

            def xslice(tb, ci):
                e = xts_of[tb]
                if isinstance(e, tuple):
                    return e[1][ci // 2][:, ci % 2, :]
                return e[:, ci, :]

            def load_xt(tb):
                xtile = xtp.tile([128, NCH, 512], bf16, tag="xt", name=f"xt_{tb}")
                nc.sync.dma_start(out=xtile[:, 0:4, :],
                                  in_=xr[:, 0:4, tb * 512:tb * 512 + 512])
                nc.sync.dma_start(out=xtile[:, 4:8, :],
                                  in_=xr[:, 4:8, tb * 512:tb * 512 + 512])
                xts_of[tb] = xtile

            # ---------------- builder pieces --------------------------------
            def proj_steps(tb, prefetch_first=True):
                """Closures for t-block tb's projections + rope + V."""
                ts = slice(tb * 512, tb * 512 + 512)
                steps = []
                state = {}

                def qk_mm(nmi, ci, nm=None):
                    if ci == 0:
                        state[nmi] = qk_ps.tile([128, 2, 512], f32, tag="qk",
                                                name=f"qk_{tb}_{nmi}")
                    p = state[nmi]
                    wt = w_qk1 if nmi == 0 else w_qk2
                    for eo in range(2):
                        nc.tensor.matmul(
                            p[:, eo, :], wt[:, ci, 128 * eo:128 * eo + 128],
                            xslice(tb, ci),
                            start=(ci == 0), stop=(ci == NCH - 1))

                def rope_muls(nmi):
                    p = state[nmi]
                    t1 = rope_t.tile([128, 512], f32, tag="t1")
                    t2 = rope_t.tile([128, 512], f32, tag="t2")
                    t3 = rope_t.tile([128, 512], f32, tag="t3")
                    t4 = rope_t.tile([128, 512], f32, tag="t4")
                    nc.vector.tensor_mul(t1, p[:, 0, :], t_cs[:, 0, ts])
                    nc.vector.tensor_mul(t2, p[:, 1, :], t_cs[:, 1, ts])
                    nc.vector.tensor_mul(t3, p[:, 1, :], t_cs[:, 0, ts])
                    nc.vector.tensor_mul(t4, p[:, 0, :], t_cs[:, 1, ts])
                    state[(nmi, "t")] = (t1, t2, t3, t4)

                def rope_addsub(nmi):
                    t1, t2, t3, t4 = state.pop((nmi, "t"))
                    pr = pairp.tile([128, 2, 512], bf16, tag="pair",
                                    name=f"pair_{tb}_{nmi}")
                    with nc.allow_low_precision(reason="bf16 attention operands"):
                        nc.vector.tensor_sub(pr[:, 0, :], t1, t2)
                        nc.vector.tensor_add(pr[:, 1, :], t3, t4)
                    state[(nmi, "pair")] = pr

                def repack(nmi, a):
                    # head a's evens/odds (rope layout, partitions 32a:+32) ->
                    # rQ/rK head layout: partitions 64*(a%2)+32*eo, slot a//2
                    pr = state[(nmi, "pair")]
                    dst = rQ if nmi == 0 else rK
                    q = nc.sync if nmi == 0 else nc.gpsimd
                    for eo in range(2):
                        q.dma_start(
                            out=dst[64 * (a % 2) + 32 * eo:
                                    64 * (a % 2) + 32 * eo + 32, a // 2, ts],
                            in_=pr[32 * a:32 * a + 32, eo, :])

                def v_mm(s):
                    pv = scr_ps.tile([128, 512], f32, tag="scr",
                                     name=f"pv_{tb}_{s}")
                    for ci in range(NCH):
                        nc.tensor.matmul(
                            pv[:, 0:OC], xslice(tb, ci)[:, 128 * s:128 * s + 128],
                            w_v[:, ci, :], start=(ci == 0), stop=(ci == NCH - 1))
                    state[("pv", s)] = pv

                def v_copy(s):
                    pv = state.pop(("pv", s))
                    pr, j = 2 * tb + s // 2, s % 2
                    with nc.allow_low_precision(reason="bf16 V"):
                        nc.scalar.copy(
                            out=Vt[:, pr, :, j, 0:D],
                            in_=pv[:, 0:OC].rearrange("p (h d) -> p h d", h=HPC))

                qk_steps = []
                if prefetch_first and tb + 1 < NT:
                    qk_steps.append(lambda: load_xt(tb + 1))
                for ci in range(NCH):
                    qk_steps.append(lambda ci=ci: qk_mm(0, ci))
                qk_steps.append(lambda: rope_muls(0))
                qk_steps.append(lambda: rope_addsub(0))
                for a in range(HPC):
                    qk_steps.append(lambda a=a: repack(0, a))
                for ci in range(NCH):
                    qk_steps.append(lambda ci=ci: qk_mm(1, ci))
                qk_steps.append(lambda: rope_muls(1))
                qk_steps.append(lambda: rope_addsub(1))
                for a in range(HPC):
                    qk_steps.append(lambda a=a: repack(1, a))
                v_steps = []
                for s in range(4):
                    v_steps.append(lambda s=s: v_mm(s))
                    v_steps.append(lambda s=s: v_copy(s))
                if not prefetch_first and tb + 1 < NT:
                    v_steps.append(lambda: load_xt(tb + 1))
                return qk_steps, v_steps

            def yproj_steps(w, pools=None, fine=False):
                """Closures for output projection of window w."""
                steps = []
                state = {"n": 0}
                pools = pools or [(scr_ps, [128, 512], "scr")]

                def py_mm(tsub, ob2):
                    if "sy" not in state:
                        state["sy"] = ysb.tile([128, 4, 2, 512], bf16, tag="sy",
                                               name=f"sy_{w}")
                    tsl = slice((4 * w + tsub) * 128, (4 * w + tsub) * 128 + 128)
                    pool, shape, tag = pools[state["n"] % len(pools)]
                    state["n"] += 1
                    py = pool.tile(shape, f32, tag=tag,
                                   name=f"py_{w}_{tsub}_{ob2}")
                    if len(shape) == 3:
                        py = py[:, 0, :]
                    for k in range(2):
                        nc.tensor.matmul(
                            py, OCt[:, k, tsl], w_o[:, k, 512 * ob2:512 * ob2 + 512],
                            start=(k == 0), stop=(k == 1))
                    with nc.allow_low_precision(reason="bf16 output"):
                        if w < 2 or (fine and (tsub + ob2) % 2 == 1):
                            nc.scalar.copy(out=state["sy"][:, tsub, ob2, :],
                                           in_=py)
                        else:
                            nc.vector.tensor_copy(
                                out=state["sy"][:, tsub, ob2, :], in_=py)

                def y_dma(tsub, ob2=None):
                    if ob2 is None:
                        nc.sync.dma_start(out=yr[:, 4 * w + tsub, :, :],
                                          in_=state["sy"][:, tsub, :, :])
                    else:
                        nc.sync.dma_start(out=yr[:, 4 * w + tsub, ob2, :],
                                          in_=state["sy"][:, tsub, ob2, :])

                for tsub in range(4):
                    for ob2 in range(2):
                        steps.append(lambda tsub=tsub, ob2=ob2: py_mm(tsub, ob2))
                        if fine:
                            steps.append(
                                lambda tsub=tsub, ob2=ob2: y_dma(tsub, ob2))
                    if not fine:
                        steps.append(lambda tsub=tsub: y_dma(tsub))
                return steps

            # ---------------- main schedule ---------------------------------
            qk0, v0 = proj_steps(0, prefetch_first=False)
            for step in qk0 + v0:
                step()

            for w in range(NT):
                chain = []
                fillers = []
                if w + 1 < NT:
                    qkn, vn = proj_steps(w + 1)
                    chain += qkn
                    fillers += vn
                if w > 0:
                    if w == NT - 1:
                        fillers += yproj_steps(w - 1, pools=[
                            (scr_ps, [128, 512], "scr"),
                            (qk_ps, [128, 2, 512], "qk")])
                    else:
                        fillers += yproj_steps(w - 1)
                qlo = 512 * w
                npair = 2 * w + 2
                slots = HPC * npair

                def pop_fillers(frac_done):
                    # qk chain front-loaded hard; bulk fillers gentler
                    want_c = int(len_c0 * min(1.0, 2.2 * frac_done) + 0.999)
                    while emitted_c[0] < want_c and chain:
                        chain.pop(0)()
                        emitted_c[0] += 1
                    want = int(len_f0 * min(1.0, 1.45 * frac_done) + 0.999)
                    while emitted[0] < want and fillers:
                        fillers.pop(0)()
                        emitted[0] += 1

                len_c0 = len(chain)
                len_f0 = len(fillers)
                emitted_c = [0]
                emitted = [0]
                slot = [0]

                for h in (1, 3, 0, 2):
                    hb, ob = 64 * (h % 2), h // 2
                    ot = ot_ps.tile([128, 512], f32, tag="ot", name=f"ot_{w}_{h}")
                    pend = []

                    def emit_pv(h=h, w=w, ot=ot):
                        # one matmul per 128-key chunk
                        p, est, lo = pend.pop(0)
                        for j in range(2):
                            nc.tensor.matmul(
                                ot[0:D + 1, lo:512], Vt[:, p, h, j, :],
                                est[:, j, lo:512],
                                start=(p == 0 and j == 0),
                                stop=(p == npair - 1 and j == 1),
                                skip_group_check=True)

                    for p in range(npair):
                        lo = 256 if p == npair - 1 else 0
                        st = st_ps.tile([128, 2, 512], f32, tag="st")
                        for j, c in enumerate((2 * p, 2 * p + 1)):
                            rk = 128 * c - qlo
                            diag = rk >= 0
                            nc.tensor.matmul(
                                st[:, j, lo:512],
                                rK[hb:hb + 64, ob, 128 * c:128 * c + 128],
                                rQ[hb:hb + 64, ob, qlo + lo:qlo + 512],
                                start=True, stop=not diag)
                            if diag:
                                ms = max(lo, rk - 128)
                                nc.tensor.matmul(
                                    st[:, j, ms:rk + 128], t_id,
                                    tri2[:, 128 - (rk - ms):256],
                                    start=False, stop=True,
                                    skip_group_check=True)
                        est = estp.tile([128, 2, 512], bf16, tag="est")
                        with nc.allow_low_precision(reason="bf16 softmax"):
                            nc.scalar.activation(out=est[:, :, lo:512],
                                                 in_=st[:, :, lo:512],
                                                 func=Exp, scale=0.125,
                                                 bias=bias_t[:, :])
                        pend.append((p, est, lo))
                        if len(pend) > 3:
                            emit_pv()
                        slot[0] += 1
                        pop_fillers(slot[0] / slots)
                    while pend:
                        emit_pv()
                    # normalize: 1/l broadcast across D partitions, scale, store
                    rl = nrm.tile([65, 512], f32r, tag="rl")
                    with nc.allow_low_precision(reason="1/l feeds matmul"):
                        nc.vector.reciprocal(out=rl[64:65, :], in_=ot[64:65, :])
                    rlb = scr_ps.tile([128, 512], f32, tag="scr",
                                      name=f"rlb_{w}_{h}")
                    nc.tensor.matmul(rlb[0:D, :], t_one[64:65, :], rl[64:65, :],
                                     start=True, stop=True)
                    rlb_sb = nrm.tile([64, 512], f32, tag="rlbsb")
                    nc.gpsimd.tensor_copy(out=rlb_sb, in_=rlb[0:D, :])
                    with nc.allow_low_precision(reason="bf16 attention out"):
                        if h % 2 == 0:
                            # partition-aligned: write OCt rows 0:64 directly
                            nc.vector.tensor_mul(
                                OCt[0:D, ob, qlo:qlo + 512], ot[0:D, :], rlb_sb)
                        else:
                            otn = nrm.tile([64, 512], bf16, tag="otn")
                            nc.vector.tensor_mul(otn, ot[0:D, :], rlb_sb)
                            nc.sync.dma_start(
                                out=OCt[64:64 + D, ob, qlo:qlo + 512], in_=otn)
                while chain:
                    chain.pop(0)()
                while fillers:
                    fillers.pop(0)()

            for step in yproj_steps(NT - 1, pools=[
                    (scr_ps, [128, 512], "scr"),
                    (st_ps, [128, 2, 512], "st"),
                    (qk_ps, [128, 2, 512], "qk")], fine=True):
                step()
    nc.compile()
    return nc


def _prep_inputs(x, wq, wk, wv, wo, rope_cos, rope_sin):
    """Host-side sharding/pre-transposition. Core i: batch i//4, head group i%4."""
    import ml_dtypes
    f = np.float32
    bf = ml_dtypes.bfloat16
    COS = np.tile(np.ascontiguousarray(rope_cos.T.astype(f)), (4, 1))  # [128,T]
    SIN = np.tile(np.ascontiguousarray(rope_sin.T.astype(f)), (4, 1))
    cs = np.stack([COS, SIN], axis=1).astype(bf)                       # [128,2,T]
    cons = np.zeros((128, 384), f)
    cons[:, 0:128] = NEG
    cons[:, 128:256] = np.where(
        np.arange(128)[:, None] > np.arange(128)[None, :], f(NEG), f(0.0))
    cons[:, 256:384] = np.eye(128, dtype=f)
    cons = cons.astype(bf)
    xT = [np.ascontiguousarray(x[b].T.astype(bf)) for b in range(B)]
    in_maps = []
    for core in range(8):
        b, g = core // 4, core % 4
        heads = [4 * g + a for a in range(HPC)]
        e_rows = np.concatenate([64 * h + 2 * np.arange(32) for h in heads])
        o_rows = e_rows + 1
        sl = slice(OC * g, OC * g + OC)
        wqkc = np.concatenate(
            [wq[e_rows].T, wq[o_rows].T, wk[e_rows].T, wk[o_rows].T],
            axis=1).astype(bf)
        in_maps.append({
            "xt": xT[b],
            "wqk": np.ascontiguousarray(wqkc),
            "wv": np.ascontiguousarray(wv[sl].T.astype(bf)),
            "wos": np.ascontiguousarray(wo[:, sl].T.astype(bf)),
            "cs": cs, "cons": cons,
            "ones": np.ones((65, 64), np.float32),
        })
    return in_maps


def kernel(x, wq, wk, wv, wo, rope_cos, rope_sin, _trace=False):
    from concourse.bass_utils import run_bass_kernel_spmd
    if "nc" not in _cache:
        _cache["nc"] = _build_nc()
    nc = _cache["nc"]
    in_maps = _prep_inputs(np.asarray(x), np.asarray(wq), np.asarray(wk),
                           np.asarray(wv), np.asarray(wo),
                           np.asarray(rope_cos), np.asarray(rope_sin))
    res = run_bass_kernel_spmd(nc, in_maps, core_ids=list(range(8)),
                               trace=_trace)
    _cache["last_result"] = res
    out = np.zeros((B, T, C), np.float32)
    for core in range(8):
        out[core // 4] += res.results[core]["y"]
    return out
